# revision 13
# baseline (speedup 1.0000x reference)
"""ABSA sparse-attention head (N=8192, D=512, degree=16, block=64) on 8 TRN2 NeuronCores.

Strategy: the adjacency built by the reference is fully structural — tokens
live in 128 blocks of 64; each token attends to a circulant band (offsets
0,±1..±4 mod 64) inside its own block plus the same-offset token in the four
blocks at positions ±2,±5 of an irrational-permutation ordering of blocks.

We order blocks by that permutation and shard 16 consecutive positions per
core with a 6-block halo on each side (28 blocks = 1792 rows per core).  All
neighbors are then local at fixed row offsets (intra: same 128-row pair;
cross: ±128 and ±320 rows), so no collectives are needed — each core computes
Q for its 1024 owned rows, K/V for all 1792 halo rows (≈6% redundant compute),
and does band attention with dense 128-wide matmuls + 0/1 masks.

Everything is bf16 on the TensorEngine (f32 accumulate in PSUM); softmax is
computed without max-subtraction (scores are O(1) for this input scale; the
1/sqrt(D) factor is folded into Wq host-side).
"""

import math
import os
import sys

import numpy as np

for _p in ("/opt/trn_rl_repo",):
    if os.path.isdir(_p) and _p not in sys.path:
        sys.path.insert(0, _p)

import ml_dtypes  # noqa: E402

# ---------------------------------------------------------------- constants
N, D = 8192, 512
BLK = 64
B = N // BLK            # 128 blocks
NCORES = 8
OWN = B // NCORES       # 16 owned blocks / core
HALO = 6                # halo blocks each side (need 5; 6 keeps 128-alignment)
LB = OWN + 2 * HALO     # 28 local blocks
LROWS = LB * BLK        # 1792 local rows
OWN0 = HALO * BLK       # first owned local row (384)
NPAIR = OWN * BLK // 128  # 8 pairs of 128 owned rows
DC = D // 128           # 4 contraction chunks
CROSS_OFFS = (-320, -128, 128, 320)  # local row offsets of the 4 cross blocks

_BF16 = ml_dtypes.bfloat16

LAST_EXEC_NS = None  # filled when tracing is enabled


def _perm():
    alpha = math.sqrt(2.0) - 1.0
    keys = np.mod(np.arange(B) * alpha, 1.0)
    return np.argsort(keys, kind="stable")


def _build_adj_expected():
    """Replica of the reference adjacency builder (for verification)."""
    degree, block, leaps = 16, BLK, (2, 5)
    P = _perm()
    P_inv = np.zeros(B, dtype=np.int64)
    P_inv[P] = np.arange(B)
    r = max(degree // 4, 1)
    cross_budget = degree - 2 * r
    n_pairs = min(2, max(cross_budget // 2, 0))
    use_leaps = list(leaps)[:n_pairs]
    b = np.arange(B)[:, None]
    o = np.arange(block)[None, :]
    cols = []
    for d in range(1, r + 1):
        cols.append(b * block + (o + d) % block)
        cols.append(b * block + (o - d) % block)
    idx = P_inv[b]
    for L in use_leaps:
        cols.append(P[(idx + L) % B] * block + o)
        cols.append(P[(idx - L) % B] * block + o)
    adj = np.stack([np.broadcast_to(c, (B, block)) for c in cols], axis=-1)
    return adj.reshape(N, -1).astype(np.int32)


def _band_mask128():
    """0/1 mask [128,128]: two 64-blocks, circulant band offsets {0,±1..±4}."""
    i = np.arange(128)[:, None]
    j = np.arange(128)[None, :]
    same = (i // BLK) == (j // BLK)
    d = (j - i) % BLK
    band = np.isin(d, [0, 1, 2, 3, 4, BLK - 4, BLK - 3, BLK - 2, BLK - 1])
    return (same & band).astype(np.float32)


def _reference_numpy(X, Wq, Wk, Wv, adj):
    """Fallback exact computation (only used if adj doesn't match structure)."""
    scale = 1.0 / math.sqrt(D)
    Q = X @ Wq.T
    K = X @ Wk.T
    V = X @ Wv.T
    nbrs = np.concatenate([adj, np.arange(N, dtype=adj.dtype)[:, None]], axis=1)
    Ki = K[nbrs]
    Vi = V[nbrs]
    scores = np.einsum("nd,nmd->nm", Q, Ki) * scale
    scores -= scores.max(axis=1, keepdims=True)
    w = np.exp(scores)
    w /= w.sum(axis=1, keepdims=True)
    return np.einsum("nm,nmd->nd", w, Vi).astype(np.float32)


# ---------------------------------------------------------------- bass graph
_GRAPH = None


def _build_graph(proj_only=False, skip_transpose=False, skip_rot=False, stage=99):
    import concourse.tile as tile
    from concourse import bacc, mybir

    bf16 = mybir.dt.bfloat16
    f32 = mybir.dt.float32
    EXP = mybir.ActivationFunctionType.Exp
    MULT = mybir.AluOpType.mult
    ADD = mybir.AluOpType.add
    AX = mybir.AxisListType.X

    nc = bacc.Bacc("TRN2", target_bir_lowering=False, debug=False,
                   num_devices=NCORES)

    xt_d = nc.dram_tensor("xt", [D, LROWS], bf16, kind="ExternalInput").ap()
    wq_d = nc.dram_tensor("wq", [D, D], bf16, kind="ExternalInput").ap()
    wk_d = nc.dram_tensor("wk", [D, D], bf16, kind="ExternalInput").ap()
    wv_d = nc.dram_tensor("wv", [D, D], bf16, kind="ExternalInput").ap()
    band_d = nc.dram_tensor("band", [128, 128], bf16, kind="ExternalInput").ap()
    iden_d = nc.dram_tensor("ident", [128, 128], bf16, kind="ExternalInput").ap()
    idm4_d = nc.dram_tensor("idm4", [128, 512], bf16, kind="ExternalInput").ap()
    out_d = nc.dram_tensor("out", [OWN * BLK, D], f32, kind="ExternalOutput").ap()

    from contextlib import ExitStack

    with tile.TileContext(nc) as tc, ExitStack() as ctx:
        sb = ctx.enter_context(tc.tile_pool(name="sb", bufs=1))
        ps = ctx.enter_context(tc.tile_pool(name="ps", bufs=8, space="PSUM"))
        wk_pool = ctx.enter_context(tc.tile_pool(name="work", bufs=3))

        # ---- persistent loads
        xt = []
        for c in range(DC):
            t = sb.tile([128, LROWS], bf16, tag=f"xt{c}", name=f"xt{c}")
            nc.sync.dma_start(out=t[:], in_=xt_d[128 * c:128 * (c + 1), :])
            xt.append(t)
        ws = {}
        for nm, dram in (("wq", wq_d), ("wk", wk_d), ("wv", wv_d)):
            ws[nm] = []
            for c in range(DC):
                t = sb.tile([128, D], bf16, tag=f"{nm}{c}", name=f"{nm}{c}")
                nc.sync.dma_start(out=t[:], in_=dram[128 * c:128 * (c + 1), :])
                ws[nm].append(t)
        band = sb.tile([128, 128], bf16, tag="band", name="band")
        nc.sync.dma_start(out=band[:], in_=band_d[:, :])
        iden = sb.tile([128, 128], bf16, tag="iden", name="iden")
        nc.sync.dma_start(out=iden[:], in_=iden_d[:, :])
        idm4 = sb.tile([128, 512], bf16, tag="idm4", name="idm4")
        nc.sync.dma_start(out=idm4[:], in_=idm4_d[:, :])

        # ---- projections
        # Q^T [dout, owned-rows]: lhsT = WqT chunk, rhs = XT chunk
        qt = [sb.tile([128, OWN * BLK], bf16, tag=f"qt{m}", name=f"qt{m}")
              for m in range(DC)]
        for m in range(DC):
            for h in range(OWN * BLK // 512):
                p = ps.tile([128, 512], f32, tag="ps", name=f"pq{m}_{h}")
                for c in range(DC):
                    nc.tensor.matmul(
                        p[:],
                        lhsT=ws["wq"][c][:, 128 * m:128 * (m + 1)],
                        rhs=xt[c][:, OWN0 + 512 * h: OWN0 + 512 * (h + 1)],
                        start=(c == 0), stop=(c == DC - 1))
                nc.any.tensor_copy(qt[m][:, 512 * h:512 * (h + 1)], p[:])

        # K^T [dout, all local rows]
        kt = [sb.tile([128, LROWS], bf16, tag=f"kt{m}", name=f"kt{m}")
              for m in range(DC)]
        spans = []
        off = 0
        while off < LROWS:
            w = min(512, LROWS - off)
            spans.append((off, w))
            off += w
        for m in range(DC):
            for (off, w) in spans:
                p = ps.tile([128, 512], f32, tag="ps", name=f"pk{m}_{off}")
                for c in range(DC):
                    nc.tensor.matmul(
                        p[:, 0:w],
                        lhsT=ws["wk"][c][:, 128 * m:128 * (m + 1)],
                        rhs=xt[c][:, off:off + w],
                        start=(c == 0), stop=(c == DC - 1))
                nc.any.tensor_copy(kt[m][:, off:off + w], p[:, 0:w])

        # V [rows, dout] (normal layout): lhsT = XT chunk (rows as M)
        vv = [sb.tile([128, D], bf16, tag=f"v{t}", name=f"v{t}")
              for t in range(LROWS // 128)]
        for t in range(LROWS // 128):
            p = ps.tile([128, 512], f32, tag="ps", name=f"pv{t}")
            for c in range(DC):
                nc.tensor.matmul(
                    p[:],
                    lhsT=xt[c][:, 128 * t:128 * (t + 1)],
                    rhs=ws["wv"][c][:, :],
                    start=(c == 0), stop=(c == DC - 1))
            nc.any.tensor_copy(vv[t][:], p[:])

        # 64-row-shifted V tiles for the ±320 spans (K=64 matmuls at
        # base_partition 64 fault the PE, so re-partition via DMA instead)
        v64 = [sb.tile([128, D], bf16, tag=f"v64_{s}", name=f"v64_{s}")
               for s in range(LROWS // 128 - 1)]
        for s in range(LROWS // 128 - 1):
            nc.sync.dma_start(out=v64[s][0:64, :], in_=vv[s][64:128, :])
            nc.sync.dma_start(out=v64[s][64:128, :], in_=vv[s + 1][0:64, :])

        # ---- attention, one 128-row pair of owned blocks at a time
        for pr in range(NPAIR if not proj_only else 0):
            r0 = OWN0 + 128 * pr      # first owned local row of this pair
            t0 = r0 // 128            # aligned V-tile index (= 3 + pr)

            sa = ps.tile([128, 128], f32, tag="ps", name=f"sa{pr}")
            sbt = ps.tile([128, 512], f32, tag="ps", name=f"sb{pr}")
            # accumulation groups sharing a PSUM bank must be sequential
            for c in range(DC):
                nc.tensor.matmul(
                    sa[:],
                    lhsT=qt[c][:, 128 * pr:128 * (pr + 1)],
                    rhs=kt[c][:, r0:r0 + 128],
                    start=(c == 0), stop=(c == DC - 1))
            for jj, offx in enumerate(CROSS_OFFS):
                for c in range(DC):
                    nc.tensor.matmul(
                        sbt[:, 128 * jj:128 * (jj + 1)],
                        lhsT=qt[c][:, 128 * pr:128 * (pr + 1)],
                        rhs=kt[c][:, r0 + offx:r0 + offx + 128],
                        start=(c == 0), stop=(c == DC - 1))

            if stage < 1:
                osb = wk_pool.tile([128, 512], f32, tag="osb", name=f"osb{pr}")
                nc.vector.tensor_copy(osb[:], sbt[:])
                nc.sync.dma_start(out=out_d[128 * pr:128 * (pr + 1), :], in_=osb[:])
                continue
            ea = wk_pool.tile([128, 128], bf16, tag="ea", name=f"ea{pr}")
            nc.scalar.activation(ea[:], sa[:], EXP)
            eb = wk_pool.tile([128, 512], bf16, tag="eb", name=f"eb{pr}")
            nc.scalar.activation(eb[:], sbt[:], EXP)

            # masked weights + row sums (fused multiply-reduce)
            if stage < 2:
                osb = wk_pool.tile([128, 512], f32, tag="osb", name=f"osb{pr}")
                nc.vector.tensor_copy(osb[:], eb[:])
                nc.sync.dma_start(out=out_d[128 * pr:128 * (pr + 1), :], in_=osb[:])
                continue
            eam = wk_pool.tile([128, 128], bf16, tag="eam", name=f"eam{pr}")
            nc.vector.tensor_tensor(out=eam[:], in0=ea[:], in1=band[:], op=MULT)
            ra = wk_pool.tile([128, 1], f32, tag="ra", name=f"ra{pr}")
            nc.vector.tensor_reduce(ra[:], eam[:], axis=AX, op=ADD)
            ebm = wk_pool.tile([128, 512], bf16, tag="ebm", name=f"ebm{pr}")
            nc.vector.tensor_tensor(out=ebm[:], in0=eb[:], in1=idm4[:], op=MULT)
            ec = wk_pool.tile([128, 4], f32, tag="ec", name=f"ec{pr}")
            nc.vector.tensor_reduce(
                ec[:], ebm[:].rearrange("p (g k) -> p g k", g=4), axis=AX, op=ADD)
            zc = wk_pool.tile([128, 1], f32, tag="zc", name=f"zc{pr}")
            nc.vector.tensor_reduce(zc[:], ec[:], axis=AX, op=ADD)
            zz = wk_pool.tile([128, 1], f32, tag="zz", name=f"zz{pr}")
            nc.vector.tensor_add(zz[:], zc[:], ra[:])
            rr = wk_pool.tile([128, 1], f32, tag="rr", name=f"rr{pr}")
            nc.vector.reciprocal(rr[:], zz[:])

            # transpose banded weights for the output matmul
            eamt = wk_pool.tile([128, 128], bf16, tag="eamt", name=f"eamt{pr}")
            if skip_transpose:
                nc.vector.tensor_copy(eamt[:], eam[:])
            else:
                tt = ps.tile([128, 128], bf16, tag="ps", name=f"tt{pr}")
                nc.tensor.transpose(tt[:], eam[:], iden[:])
                nc.vector.tensor_copy(eamt[:], tt[:])

            if stage < 3:
                osb = wk_pool.tile([128, 512], f32, tag="osb", name=f"osb{pr}")
                nc.vector.tensor_scalar_mul(osb[:], sbt[:], rr[:, 0:1])
                nc.sync.dma_start(out=out_d[128 * pr:128 * (pr + 1), :], in_=osb[:])
                continue
            # diag(exp cross score) tiles
            dgs = []
            for jj in range(4):
                dg = wk_pool.tile([128, 128], bf16, tag=f"dg{jj}",
                                  name=f"dg{pr}_{jj}")
                nc.vector.tensor_scalar_mul(dg[:], iden[:], ec[:, jj:jj + 1])
                dgs.append(dg)

            # output accumulation
            oo = ps.tile([128, 512], f32, tag="ps", name=f"oo{pr}")
            nc.tensor.matmul(oo[:], lhsT=eamt[:], rhs=vv[t0][:],
                             start=True, stop=False)
            nc.tensor.matmul(oo[:], lhsT=dgs[1][:], rhs=vv[t0 - 1][:],
                             start=False, stop=False)
            nc.tensor.matmul(oo[:], lhsT=dgs[2][:], rhs=vv[t0 + 1][:],
                             start=False, stop=False)
            # ±320-row spans via the 64-row-shifted V tiles
            nc.tensor.matmul(oo[:], lhsT=dgs[0][:], rhs=v64[pr][:],
                             start=False, stop=False)
            nc.tensor.matmul(oo[:], lhsT=dgs[3][:], rhs=v64[pr + 5][:],
                             start=False, stop=True)

            osb = wk_pool.tile([128, 512], f32, tag="osb", name=f"osb{pr}")
            nc.vector.tensor_scalar_mul(osb[:], oo[:], rr[:, 0:1])
            nc.sync.dma_start(out=out_d[128 * pr:128 * (pr + 1), :], in_=osb[:])

        if proj_only:
            for pr in range(NPAIR):
                osb = wk_pool.tile([128, 512], f32, tag="osb", name=f"osb{pr}")
                nc.vector.tensor_copy(osb[:], vv[3 + pr][:])
                nc.sync.dma_start(out=out_d[128 * pr:128 * (pr + 1), :], in_=osb[:])

    nc.compile()
    return nc


def _get_graph():
    global _GRAPH
    if _GRAPH is None:
        _GRAPH = _build_graph()
    return _GRAPH


def _install_ntff_shim():
    """Make run_bass_kernel_spmd(trace=True) work under axon in this image."""
    import types
    if "antenv.axon_hooks" not in sys.modules:
        mod = types.ModuleType("antenv.axon_hooks")
        mod._hook = None
        mod.set_axon_ntff_profile_hook = lambda h: setattr(mod, "_hook", h)
        mod.get_axon_ntff_profile_hook = lambda: mod._hook
        sys.modules["antenv.axon_hooks"] = mod
        try:
            import antenv
            antenv.axon_hooks = mod
        except ImportError:
            pass
    m = sys.modules["antenv.axon_hooks"]
    if m.get_axon_ntff_profile_hook() is None:
        try:
            from trn_agent_boot.trn_boot import _ntff_profile_via_ctypes
            m.set_axon_ntff_profile_hook(
                _ntff_profile_via_ctypes("/opt/axon/libaxon_pjrt.so"))
        except Exception:
            pass


# ---------------------------------------------------------------- entry point
def kernel(X, Wq, Wk, Wv, adj, _trace=False):
    global LAST_EXEC_NS
    X = np.asarray(X, dtype=np.float32)
    Wq = np.asarray(Wq, dtype=np.float32)
    Wk = np.asarray(Wk, dtype=np.float32)
    Wv = np.asarray(Wv, dtype=np.float32)
    adj = np.asarray(adj)

    if X.shape != (N, D) or not np.array_equal(adj, _build_adj_expected()):
        # unexpected structure: fall back to exact host computation
        return _reference_numpy(X, Wq, Wk, Wv, adj)

    P = _perm()
    scale = 1.0 / math.sqrt(D)
    wq_h = np.ascontiguousarray(Wq.T * scale).astype(_BF16)
    wk_h = np.ascontiguousarray(Wk.T).astype(_BF16)
    wv_h = np.ascontiguousarray(Wv.T).astype(_BF16)
    band_h = _band_mask128().astype(_BF16)
    iden_h = np.eye(128, dtype=np.float32).astype(_BF16)

    X_blocks = X.reshape(B, BLK, D)
    in_maps = []
    for j in range(NCORES):
        pos = np.arange(OWN * j - HALO, OWN * j + OWN + HALO) % B
        blocks = P[pos]                                # 28 block ids
        Xs = X_blocks[blocks].reshape(LROWS, D)        # [1792, 512]
        xt_h = np.ascontiguousarray(Xs.T).astype(_BF16)
        in_maps.append({
            "xt": xt_h, "wq": wq_h, "wk": wk_h, "wv": wv_h,
            "band": band_h, "ident": iden_h,
            "idm4": np.tile(np.eye(128, dtype=np.float32), (1, 4)).astype(_BF16),
        })

    trace = _trace or os.environ.get("BASS_KERNEL_TRACE") == "1"
    if trace:
        _install_ntff_shim()

    from concourse.bass_utils import run_bass_kernel_spmd
    nc = _get_graph()
    res = run_bass_kernel_spmd(nc, in_maps, list(range(NCORES)), trace=trace)
    LAST_EXEC_NS = res.exec_time_ns

    out = np.empty((B, BLK, D), dtype=np.float32)
    for j in range(NCORES):
        blocks = P[np.arange(OWN * j, OWN * (j + 1))]
        out[blocks] = res.results[j]["out"].reshape(OWN, BLK, D)
    return out.reshape(N, D)
